# revision 35
# baseline (speedup 1.0000x reference)
"""CPAB warp kernel for Trainium2, 8-core data-parallel.

Math: theta = mean_S(input_seq) @ W_loc + b_loc; A = basis @ theta -> per-cell
affine velocity v(x) = a_c x + b_c (continuous PWL, 64 cells); gamma = 50 Euler
steps of x += v(x)*dt from the uniform grid (S=4096 points in [0,1]).

Facts this kernel exploits (verified against the reference numerics):
 - Cell boundaries fall exactly at s = 64*c; max total drift is ~4.8 grid
   spacings, so only the E=8 outermost points per cell side can ever cross a
   cell boundary, and never more than one boundary.
 - 50 Euler steps of an affine field agree with the exact flow
   x(t) = x0*e^{at} + (b/a)(e^{at}-1) to ~4e-6 relative ((1+a*dt)^50 vs e^a
   with |a| <~ 0.02), far below the bf16 table noise, so bulk points use the
   closed form x1 = G*x0 + H with G = 1+a*E(a), H = b*E(a),
   E(a) = (e^a-1)/a evaluated as a cubic polynomial (error ~1e-9).
 - Edge points (in one-sided w = sigma*x coords, sigma = -1 on the left side)
   either stay in-cell (x1 = G*w0 + H) or cross the knot Tw once at
   t* = (dxT/v0)*ln1p(q)/q, q = a*dxT/v0 (2-term polynomial, |q| <~ 0.02 for
   any point that actually crosses), then flow in the neighbor cell:
   x1 = Tw + (1-t*)*v'(Tw)*E(a'*(1-t*)). Branch select by x1_nocross > Tw;
   the continuous field makes near-threshold misclassification harmless.

Engine split: DVE does a binary-tree reduce of the streamed input (the 34us
roofline pass over 16.8MB) plus ~25 closed-form edge ops; PE does the mean
partition-sum, theta/A, selector and table-expansion matmuls (bf16 operands
where tf32-level truncation is provably harmless, fp32 for G/H); ACT handles
PSUM->SBUF copies and scalar-affine table ops. Input rows stream as
contiguous-per-partition half-row DMAs issued upfront; all constants arrive
in one packed DMA; gamma leaves in one contiguous [128 x 1KB] store.

Layout: 8 rows/core, edge tile [128, 8, 8]: partition p = 16*r + cq (cq =
cell quad), free = (c4, side, e), cell c = 4*cq + c4. Per-(row,cell) tables
are expanded into this layout by +-1 selector matmuls on PE writing PSUM at
partition offset 32g via tile_position.
"""

import numpy as np


B, S, D = 64, 4096, 128
NCELLS = 64
NSTEPS = 50
DT = 1.0 / NSTEPS
DTH = NCELLS - 1  # 63
NCORES = 8
R = B // NCORES  # 8 rows per core
NPASS = R // 2  # 4 passes of 2 rows
E = 8  # edge points per cell side
NT = S // 128  # 32 blocks of 128 grid points per row

# packed constant-block column offsets (f32 words; *_B regions hold packed bf16)
C_BLOC = 0  # [63p, 1]
C_ONES = C_BLOC + 1  # [128, 1] = 1/S
C_TKP = C_ONES + 1  # [128, 1] = (c+1)/64, c = p%64
C_TKM = C_TKP + 1  # [128, 1] = c/64
C_EABS = C_TKM + 1  # [128, 8*32] f32 (finals expansion)
C_W0 = C_EABS + 8 * 32  # [128, 8*8] edge-layout w0
C_X0 = C_W0 + 8 * E  # [128, 4*64] gamma-layout grid
C_WLOCB = C_X0 + 4 * 64  # [128, 32] = 63 bf16 + pad
C_BASISTB = C_WLOCB + 32  # [63p, 64] = 128 bf16
C_SELB = C_BASISTB + 64  # [128, 256] = 8*64 bf16 selectors
C_ESGNB = C_SELB + 256  # [128, 128] = 256 bf16
C_ONESB = C_ESGNB + 128  # [128, 1] = bf16(1/S) packed
C_TW = C_ONESB + 1  # [128, 8] knot in w-coords per (p, ch)
C_SGN = C_TW + 8  # [128, 8] side sign per ch
C_DXT = C_SGN + 8  # [128, 64] Tw - w0 per (p, ch, e)
C_ESGNF = C_DXT + 64  # [128, 256] f32 signed expansion selector (unused)
C_EABSB = C_ESGNF + 256  # [128, 128] = 256 bf16 unsigned expansion selector
CCOLS = C_EABSB + 128

_CACHE = {}


def _build_program():
    import concourse.bass as bass
    import concourse.bacc as bacc
    import concourse.tile as tile
    from concourse import mybir

    alu = mybir.AluOpType
    f32 = mybir.dt.float32

    nc = bacc.Bacc("TRN2", target_bir_lowering=False, debug=False, enable_asserts=False)

    bf16 = mybir.dt.bfloat16

    def mm(out, lhsT, rhs, start=True, stop=True):
        nc.tensor.matmul(out, lhsT, rhs, start=start, stop=stop)

    seq = nc.dram_tensor("seq", [R, S, D], f32, kind="ExternalInput").ap()
    cbd = nc.dram_tensor("cb", [128, CCOLS], f32, kind="ExternalInput").ap()
    gamma = nc.dram_tensor("gamma", [R, S], f32, kind="ExternalOutput").ap()

    with tile.TileContext(nc) as tc:
        with (
            tc.tile_pool(name="const", bufs=1) as p_const,
            tc.tile_pool(name="seqp", bufs=24) as p_seq,
            tc.tile_pool(name="redp", bufs=4) as p_red,
            tc.tile_pool(name="meanps", bufs=1, space=bass.MemorySpace.PSUM) as p_mps,
            tc.tile_pool(name="passps", bufs=1, space=bass.MemorySpace.PSUM) as p_pps,
            tc.tile_pool(name="cwtps", bufs=1, space=bass.MemorySpace.PSUM) as p_cps,
            tc.tile_pool(name="sb", bufs=1) as p_sb,
            tc.tile_pool(name="tbl", bufs=1) as p_tbl,
            tc.tile_pool(name="integ", bufs=2) as p_int,
        ):
            cb = p_const.tile([128, CCOLS], f32, tag="cb")
            nc.scalar.dma_start(cb[:], cbd)
            wloc = cb[:, C_WLOCB : C_WLOCB + 32].bitcast(bf16)[:, 0:DTH]
            basisT = cb[0:DTH, C_BASISTB : C_BASISTB + 64].bitcast(bf16)
            sel_bf = cb[:, C_SELB : C_SELB + 256].bitcast(bf16)
            esgn_bf = cb[:, C_ESGNB : C_ESGNB + 128].bitcast(bf16)
            eabs_bf = cb[:, C_EABSB : C_EABSB + 128].bitcast(bf16)
            bloc = cb[0:DTH, C_BLOC : C_BLOC + 1]
            ones_bf = cb[:, C_ONESB : C_ONESB + 1].bitcast(bf16)[:, 0:1]
            w0v = cb[:, C_W0 : C_W0 + 8 * E].rearrange("p (c e) -> p c e", e=E)
            Twv = cb[:, C_TW : C_TW + 8].rearrange("p (c o) -> p c o", o=1)
            sgnv = cb[:, C_SGN : C_SGN + 8].rearrange("p (c o) -> p c o", o=1)
            dxTv = cb[:, C_DXT : C_DXT + 64].rearrange("p (c e) -> p c e", e=E)
            x0v = cb[:, C_X0 : C_X0 + 4 * 64].rearrange("p (c j) -> p c j", j=64)

            ACT_COPY = mybir.ActivationFunctionType.Copy

            mean_ps = p_mps.tile([128, R], f32, tag="meanps")
            mean_sb = p_sb.tile([128, R], bf16, tag="mean")
            # expanded per-(row,cell,side) tables in edge layout:
            # cols 0 a | 1 a' | 2 G=e^a | 3 sigma*b | 4 sigma*b' | 5 H=sigma*h
            cwt_all = p_sb.tile([128, 8, 6], f32, tag="cwtall")
            cw_ps_all = p_cps.tile([128, 8, 64], f32, tag="cwps")  # pad to a full 2KB bank

            # issue all quarter-row DMAs upfront (contiguous 4KB/partition
            # each); the pool's WAR tracking paces reuse automatically
            NQ = NT // 4  # 8 blocks per quarter
            seq_qs = []
            for i in range(4 * R):
                t = p_seq.tile([128, NQ, D], f32, tag="seq", name=f"seq{i}")
                nc.sync.dma_start(
                    t[:],
                    seq[i // 4].rearrange("(h p n) d -> h p n d", h=4, p=128)[i % 4],
                )
                seq_qs.append(t)

            def quarter_tree(r, h):
                # one DVE halving add down to [128, 4, D] bf16; PE (FWL
                # matmuls) finishes the sum
                src = seq_qs[4 * r + h]
                m = NQ // 2
                dst = p_red.tile(
                    [128, m, D], bf16, tag=f"red{h}", name=f"red{r}_{h}"
                )
                nc.vector.tensor_tensor(
                    out=dst[:], in0=src[:, 0:m, :], in1=src[:, m : 2 * m, :],
                    op=alu.add,
                )
                return dst

            def do_row(r):
                qs = [quarter_tree(r, h) for h in range(4)]
                k = 0
                for t in qs:
                    for j in range(4):
                        mm(mean_ps[:, r : r + 1], t[:, j, :], ones_bf,
                           start=(k == 0), stop=(k == 15))
                        k += 1
                nc.scalar.copy(mean_sb[:, r : r + 1], mean_ps[:, r : r + 1])

            def do_pass(g):
                # theta & A for rows (2g, 2g+1)
                th_ps = p_pps.tile([DTH, 2], f32, tag="thps", name=f"thps{g}")
                mm(th_ps[:], wloc, mean_sb[:, 2 * g : 2 * g + 2])
                th = p_tbl.tile([DTH, 2], bf16, tag=f"th{g}")
                nc.vector.tensor_scalar(
                    out=th[:], in0=th_ps[:], scalar1=bloc, scalar2=None, op0=alu.add
                )
                ab_ps = p_pps.tile([128, 2], f32, tag="abps", name=f"abps{g}")
                mm(ab_ps[:], basisT, th[:])
                ab = p_tbl.tile([128, 2], bf16, tag=f"ab{g}")
                nc.scalar.copy(ab[:], ab_ps[:])

                # per-(h,c) constants:
                # q = (a_cur, a_cur, a_nxt, a_prv, b_cur, b_cur, b_nxt, b_prv)
                c_ps = p_pps.tile([128, 8], f32, tag="cps", name=f"cps{g}")
                for q in range(8):
                    for h in range(2):
                        mm(
                            c_ps[64 * h : 64 * h + 64, q : q + 1],
                            sel_bf[:, 64 * q : 64 * q + 64],
                            ab[:, h : h + 1],
                        )
                cons = p_tbl.tile([128, 8], f32, tag=f"cons{g}")
                nc.scalar.copy(cons[:], c_ps[:])
                a_cur, b_cur = cons[:, 0:1], cons[:, 4:5]

                # TBf: eabs cols 0:6 = (a,a,a',a'_other,G,G),
                #      esgn cols 6:12 = (b,b,b'n,b'p,h,h); stride-2 side picks
                TBf = p_tbl.tile([128, 12], bf16, tag=f"TBf{g}")
                nc.scalar.copy(TBf[:, 0:4], cons[:, 0:4])
                nc.scalar.copy(TBf[:, 6:10], cons[:, 4:8])
                # E(a) = (e^a - 1)/a = 1 + a(1/2 + a(1/6 + a/24)); g = 1 + aE, h = bE
                ep = p_tbl.tile([128, 2], f32, tag=f"ep{g}")
                nc.vector.tensor_scalar(
                    out=ep[:, 0:1], in0=a_cur, scalar1=float(1.0 / 24.0),
                    scalar2=float(1.0 / 6.0), op0=alu.mult, op1=alu.add,
                )
                nc.vector.tensor_scalar(
                    out=ep[:, 1:2], in0=ep[:, 0:1], scalar1=a_cur,
                    scalar2=0.5, op0=alu.mult, op1=alu.add,
                )
                Ea = p_tbl.tile([128, 1], f32, tag=f"Ea{g}")
                nc.vector.tensor_scalar(
                    out=Ea[:], in0=ep[:, 1:2], scalar1=a_cur,
                    scalar2=1.0, op0=alu.mult, op1=alu.add,
                )
                nc.vector.tensor_scalar(
                    out=TBf[:, 4:5], in0=Ea[:], scalar1=a_cur, scalar2=None,
                    op0=alu.mult,
                )  # G-1 = a*E(a): bf16-safe (small), reconstruct G-1+1 downstream
                nc.scalar.copy(TBf[:, 5:6], TBf[:, 4:5])
                nc.vector.tensor_scalar(
                    out=TBf[:, 10:11], in0=Ea[:], scalar1=b_cur, scalar2=None,
                    op0=alu.mult,
                )
                nc.scalar.copy(TBf[:, 11:12], TBf[:, 10:11])

                # expansion into edge layout (PE), directly at partition 32g
                cw_ps = cw_ps_all[32 * g : 32 * g + 32]
                for ch in range(8):
                    side = ch % 2  # 0=L, 1=R
                    base = 1 - side
                    nc.tensor.matmul(
                        cw_ps[:, ch, 0:3],
                        eabs_bf[:, 32 * ch : 32 * ch + 32],
                        TBf[:, 0:6].rearrange("p (a b) -> p a b", b=2)[
                            :, :, base : base + 1
                        ],
                        start=True, stop=True, tile_position=(0, 32 * g),
                    )
                    nc.tensor.matmul(
                        cw_ps[:, ch, 3:6],
                        esgn_bf[:, 32 * ch : 32 * ch + 32],
                        TBf[:, 6:12].rearrange("p (a b) -> p a b", b=2)[
                            :, :, base : base + 1
                        ],
                        start=True, stop=True, tile_position=(0, 32 * g),
                    )
                nc.scalar.copy(cwt_all[32 * g : 32 * g + 32, :, :], cw_ps[:, :, 0:6])

            for r in range(R):
                do_row(r)
                if r % 2 == 1:
                    do_pass(r // 2)

            # ---- bulk finals: x = x0 + ((G-1)*x0 + H) ----
            Gm1b = cwt_all[:, 1::2, 2:3]  # [128, 4, 1] (side-independent)
            Hb = cwt_all[:, 1::2, 5:6]  # +h on the right-side channels
            xg1 = p_int.tile([128, 4, 64], f32, tag="xg1")
            nc.vector.tensor_tensor(
                out=xg1[:], in0=x0v, in1=Gm1b.broadcast_to([128, 4, 64]), op=alu.mult
            )
            nc.vector.tensor_tensor(
                out=xg1[:], in0=xg1[:], in1=Hb.broadcast_to([128, 4, 64]), op=alu.add
            )
            xg = p_int.tile([128, 4, 64], f32, tag="xg")
            nc.vector.tensor_tensor(
                out=xg[:], in0=xg1[:], in1=x0v, op=alu.add
            )

            # ---- edge points: exact affine-flow closed form in w-coords ----
            # own-cell flow: w_nc = G*w0 + Hw; crossing iff w_nc > Tw;
            # crossing time t* = (dxT/v0)*ln1p(q)/q with q = a*dxT/v0;
            # then x1 = Tw + (1-t*)*v'(Tw)*E(a'*(1-t*)) in the neighbor cell
            SH = [128, 8, E]
            ab_ = cwt_all[:, :, 0:1].broadcast_to(SH)
            apb = cwt_all[:, :, 1:2].broadcast_to(SH)
            Gb2 = cwt_all[:, :, 2:3].broadcast_to(SH)
            bwb = cwt_all[:, :, 3:4].broadcast_to(SH)
            Hwb = cwt_all[:, :, 5:6].broadcast_to(SH)
            Twb = Twv.broadcast_to(SH)
            sgb = sgnv.broadcast_to(SH)

            def etile(nm):
                return p_int.tile(SH, f32, tag=nm, name=nm)

            # v'(Tw) in the neighbor cell, per (p, ch)
            vtp = p_int.tile([128, 8, 1], f32, tag="vtp")
            nc.vector.tensor_tensor(
                out=vtp[:], in0=cwt_all[:, :, 1:2], in1=Twv, op=alu.mult
            )
            nc.vector.tensor_tensor(
                out=vtp[:], in0=vtp[:], in1=cwt_all[:, :, 4:5], op=alu.add
            )
            vtpb = vtp[:].broadcast_to(SH)

            v0 = etile("v0")
            nc.vector.tensor_tensor(out=v0[:], in0=w0v, in1=ab_, op=alu.mult)
            nc.vector.tensor_tensor(out=v0[:], in0=v0[:], in1=bwb, op=alu.add)
            nc.vector.tensor_scalar(
                out=v0[:], in0=v0[:], scalar1=1e-12, scalar2=None, op0=alu.add
            )
            rv = etile("rv")
            nc.vector.reciprocal(rv[:], v0[:])
            nc.vector.tensor_scalar(
                out=rv[:], in0=rv[:], scalar1=1e6, scalar2=-1e6,
                op0=alu.min, op1=alu.max,
            )
            q = etile("q")
            nc.vector.tensor_tensor(out=q[:], in0=rv[:], in1=ab_, op=alu.mult)
            nc.vector.tensor_tensor(out=q[:], in0=q[:], in1=dxTv, op=alu.mult)
            u = etile("u")
            nc.vector.tensor_tensor(out=u[:], in0=rv[:], in1=dxTv, op=alu.mult)
            L = etile("L")
            nc.vector.tensor_scalar(
                out=L[:], in0=q[:], scalar1=float(1.0 / 3.0), scalar2=-0.5,
                op0=alu.mult, op1=alu.add,
            )
            nc.vector.tensor_tensor(out=L[:], in0=L[:], in1=q[:], op=alu.mult)
            nc.vector.tensor_scalar(
                out=L[:], in0=L[:], scalar1=1.0, scalar2=None, op0=alu.add
            )
            tau = etile("tau")
            nc.vector.tensor_tensor(out=tau[:], in0=u[:], in1=L[:], op=alu.mult)
            nc.vector.tensor_scalar(
                out=tau[:], in0=tau[:], scalar1=-1.0, scalar2=1.0,
                op0=alu.mult, op1=alu.add,
            )
            m = etile("m")
            nc.vector.tensor_tensor(out=m[:], in0=tau[:], in1=apb, op=alu.mult)
            Em = etile("Em")
            nc.vector.tensor_scalar(
                out=Em[:], in0=m[:], scalar1=float(1.0 / 6.0), scalar2=0.5,
                op0=alu.mult, op1=alu.add,
            )
            nc.vector.tensor_tensor(out=Em[:], in0=Em[:], in1=m[:], op=alu.mult)
            nc.vector.tensor_scalar(
                out=Em[:], in0=Em[:], scalar1=1.0, scalar2=None, op0=alu.add
            )
            xc = etile("xc")
            nc.vector.tensor_tensor(out=xc[:], in0=tau[:], in1=vtpb, op=alu.mult)
            nc.vector.tensor_tensor(out=xc[:], in0=xc[:], in1=Em[:], op=alu.mult)
            nc.vector.tensor_tensor(out=xc[:], in0=xc[:], in1=Twb, op=alu.add)
            xnc = etile("xnc")
            nc.vector.tensor_tensor(out=xnc[:], in0=w0v, in1=Gb2, op=alu.mult)
            nc.vector.tensor_tensor(out=xnc[:], in0=xnc[:], in1=Hwb, op=alu.add)
            nc.vector.tensor_tensor(out=xnc[:], in0=xnc[:], in1=w0v, op=alu.add)
            sel = etile("sel")
            nc.vector.tensor_tensor(out=sel[:], in0=xnc[:], in1=Twb, op=alu.is_gt)
            dd = etile("dd")
            nc.vector.tensor_tensor(out=dd[:], in0=xc[:], in1=xnc[:], op=alu.subtract)
            nc.vector.tensor_tensor(out=dd[:], in0=dd[:], in1=sel[:], op=alu.mult)
            xe = etile("xe")
            nc.vector.tensor_tensor(out=xe[:], in0=xnc[:], in1=dd[:], op=alu.add)
            nc.vector.tensor_tensor(out=xe[:], in0=xe[:], in1=sgb, op=alu.mult)

            # merge edges into the gamma tile
            nc.vector.tensor_copy(xg[:, :, 0:E], xe[:, 0:8:2, :])
            nc.vector.tensor_copy(xg[:, :, 64 - E : 64], xe[:, 1:8:2, :])
            nc.sync.dma_start(
                gamma.rearrange("r (cq f) -> (r cq) f", f=256), xg[:]
            )

    nc.compile()
    return nc


def _pack_bf16(a):
    """(P, n) float -> (P, ceil(n/2)) float32 words holding packed bf16 pairs."""
    v = np.ascontiguousarray(a, dtype=np.float32).view(np.uint32)
    # round-to-nearest-even truncation to bf16
    a16 = ((v + 0x7FFF + ((v >> 16) & 1)) >> 16).astype(np.uint16)
    P, n = a16.shape
    if n % 2:
        a16 = np.concatenate([a16, np.zeros((P, 1), np.uint16)], axis=1)
    u32 = a16[:, 0::2].astype(np.uint32) | (a16[:, 1::2].astype(np.uint32) << 16)
    return u32.view(np.float32)


def _host_constants():
    f32 = np.float32
    grid = np.linspace(0.0, 1.0, S).astype(f32)
    cbk = np.zeros((128, CCOLS), dtype=f32)
    c = np.arange(128, dtype=np.int64) % 64
    cbk[:, C_ONES] = 1.0 / S  # 2^-12, exact
    cbk[:, C_ONESB : C_ONESB + 1] = _pack_bf16(np.full((128, 1), 1.0 / S, f32))
    cbk[:, C_TKP] = ((c + 1) / 64.0).astype(f32)
    cbk[:, C_TKM] = (c / 64.0).astype(f32)
    # selectors: row k = flat A index, col = q*64 + cell
    # q = (a_cur, a_cur, a_nxt, a_prv, b_cur, b_cur, b_nxt, b_prv)
    sel = np.zeros((128, 8 * 64), dtype=f32)
    cc = np.arange(64)
    sel[2 * cc, 0 * 64 + cc] = 1.0
    sel[2 * cc, 1 * 64 + cc] = 1.0
    sel[np.minimum(2 * cc + 2, 126), 2 * 64 + cc] = 1.0  # a_nxt (c=63 -> self)
    sel[np.maximum(2 * cc - 2, 0), 3 * 64 + cc] = 1.0  # a_prv (c=0 -> self)
    sel[2 * cc + 1, 4 * 64 + cc] = 1.0
    sel[2 * cc + 1, 5 * 64 + cc] = 1.0
    sel[np.minimum(2 * cc + 3, 127), 6 * 64 + cc] = 1.0  # b_nxt (c=63 -> self)
    sel[np.maximum(2 * cc - 1, 1), 7 * 64 + cc] = 1.0  # b_prv (c=0 -> self)
    cbk[:, C_SELB : C_SELB + 256] = _pack_bf16(sel)
    # expansion selectors: k = h*64 + c (pass layout), m = 16*h + cq (local)
    esgn = np.zeros((128, 8 * 32), dtype=f32)
    eabs = np.zeros((128, 8 * 32), dtype=f32)
    for ch in range(8):
        c4, side = ch // 2, ch % 2
        sgn = -1.0 if side == 0 else 1.0
        for m in range(32):
            h, cq = m // 16, m % 16
            k = h * 64 + 4 * cq + c4
            esgn[k, 32 * ch + m] = sgn
            eabs[k, 32 * ch + m] = 1.0
    cbk[:, C_ESGNB : C_ESGNB + 128] = _pack_bf16(esgn)
    cbk[:, C_EABSB : C_EABSB + 128] = _pack_bf16(eabs)
    cbk[:, C_EABS : C_EABS + 256] = eabs
    # w0[p, ch, e]: p = 16r + cq, ch = (c4, side); L: -grid[64c+e], R: grid[64c+56+e]
    w0map = np.zeros((128, 8, E), dtype=f32)
    for p in range(128):
        cq = p % 16
        for ch in range(8):
            c4, side = ch // 2, ch % 2
            cell = 4 * cq + c4
            if side == 0:
                w0map[p, ch, :] = -grid[64 * cell : 64 * cell + E]
            else:
                w0map[p, ch, :] = grid[64 * cell + 64 - E : 64 * cell + 64]
    cbk[:, C_W0 : C_W0 + 64] = w0map.reshape(128, 64)
    # knot in w-coords, side sign, and Tw - w0 per (p, ch[, e])
    Tw = np.zeros((128, 8), dtype=f32)
    sgn = np.zeros((128, 8), dtype=f32)
    for p in range(128):
        cq = p % 16
        for ch in range(8):
            c4, side = ch // 2, ch % 2
            cell = 4 * cq + c4
            if side == 0:
                Tw[p, ch] = -cell / 64.0
                sgn[p, ch] = -1.0
            else:
                Tw[p, ch] = (cell + 1) / 64.0
                sgn[p, ch] = 1.0
    cbk[:, C_TW : C_TW + 8] = Tw
    cbk[:, C_SGN : C_SGN + 8] = sgn
    cbk[:, C_DXT : C_DXT + 64] = (Tw[:, :, None] - w0map).reshape(128, 64)
    # x0 in gamma layout: [p=(r,cq), c4, j] = grid[256*cq + 64*c4 + j]
    cq = np.arange(128) % 16
    x0g = grid[
        (256 * cq)[:, None, None]
        + (64 * np.arange(4))[None, :, None]
        + np.arange(64)[None, None, :]
    ]
    cbk[:, C_X0 : C_X0 + 256] = x0g.reshape(128, 256)
    return cbk


def _in_map(input_seq_slice, W_loc, b_loc, basis, cbk):
    f32 = np.float32
    cbk = cbk.copy()
    cbk[:, C_WLOCB : C_WLOCB + 32] = _pack_bf16(np.asarray(W_loc, dtype=f32))
    cbk[0:DTH, C_BASISTB : C_BASISTB + 64] = _pack_bf16(
        np.asarray(basis, dtype=f32).T
    )
    cbk[0:DTH, C_BLOC] = np.asarray(b_loc, dtype=f32)
    return {
        "seq": np.ascontiguousarray(input_seq_slice, dtype=f32),
        "cb": cbk,
    }


def kernel(input_seq, W_loc, b_loc, basis):
    from concourse.bass_utils import run_bass_kernel_spmd

    if "nc" not in _CACHE:
        _CACHE["nc"] = _build_program()
    nc = _CACHE["nc"]
    cbk = _host_constants()
    in_maps = [
        _in_map(input_seq[k * R : (k + 1) * R], W_loc, b_loc, basis, cbk)
        for k in range(NCORES)
    ]
    res = run_bass_kernel_spmd(nc, in_maps, core_ids=list(range(NCORES)))
    return np.concatenate([r["gamma"] for r in res.results], axis=0)


# revision 36
# speedup vs baseline: 1.0998x; 1.0998x over previous
"""CPAB warp kernel for Trainium2, 8-core data-parallel.

Math: theta = mean_S(input_seq) @ W_loc + b_loc; A = basis @ theta -> per-cell
affine velocity v(x) = a_c x + b_c (continuous PWL, 64 cells); gamma = 50 Euler
steps of x += v(x)*dt from the uniform grid (S=4096 points in [0,1]).

Facts this kernel exploits (verified against the reference numerics):
 - Cell boundaries fall exactly at s = 64*c; max total drift is ~4.8 grid
   spacings, so only the E=8 outermost points per cell side can ever cross a
   cell boundary, and never more than one boundary.
 - 50 Euler steps of an affine field agree with the exact flow
   x(t) = x0*e^{at} + (b/a)(e^{at}-1) to ~4e-6 relative ((1+a*dt)^50 vs e^a
   with |a| <~ 0.02), far below the bf16 table noise, so bulk points use the
   closed form x1 = G*x0 + H with G = 1+a*E(a), H = b*E(a),
   E(a) = (e^a-1)/a evaluated as a cubic polynomial (error ~1e-9).
 - Edge points (in one-sided w = sigma*x coords, sigma = -1 on the left side)
   either stay in-cell (x1 = G*w0 + H) or cross the knot Tw once at
   t* = (dxT/v0)*ln1p(q)/q, q = a*dxT/v0 (2-term polynomial, |q| <~ 0.02 for
   any point that actually crosses), then flow in the neighbor cell:
   x1 = Tw + (1-t*)*v'(Tw)*E(a'*(1-t*)). Branch select by x1_nocross > Tw;
   the continuous field makes near-threshold misclassification harmless.

Engine split: DVE does a binary-tree reduce of the streamed input (the 34us
roofline pass over 16.8MB) plus ~25 closed-form edge ops; PE does the mean
partition-sum, theta/A, selector and table-expansion matmuls (bf16 operands
where tf32-level truncation is provably harmless, fp32 for G/H); ACT handles
PSUM->SBUF copies and scalar-affine table ops. Input rows stream as
contiguous-per-partition half-row DMAs issued upfront; all constants arrive
in one packed DMA; gamma leaves in one contiguous [128 x 1KB] store.

Layout: 8 rows/core, edge tile [128, 8, 8]: partition p = 16*r + cq (cq =
cell quad), free = (c4, side, e), cell c = 4*cq + c4. Per-(row,cell) tables
are expanded into this layout by +-1 selector matmuls on PE writing PSUM at
partition offset 32g via tile_position.
"""

import numpy as np


B, S, D = 64, 4096, 128
NCELLS = 64
NSTEPS = 50
DT = 1.0 / NSTEPS
DTH = NCELLS - 1  # 63
NCORES = 8
R = B // NCORES  # 8 rows per core
NPASS = R // 2  # 4 passes of 2 rows
E = 8  # edge points per cell side
NT = S // 128  # 32 blocks of 128 grid points per row

# packed constant-block column offsets (f32 words; *_B regions hold packed bf16)
C_BLOC = 0  # [63p, 1]
C_ONES = C_BLOC + 1  # [128, 1] = 1/S
C_TKP = C_ONES + 1  # [128, 1] = (c+1)/64, c = p%64
C_TKM = C_TKP + 1  # [128, 1] = c/64
C_EABS = C_TKM + 1  # [128, 8*32] f32 (finals expansion)
C_W0 = C_EABS + 8 * 32  # [128, 8*8] edge-layout w0
C_X0 = C_W0 + 8 * E  # [128, 4*64] gamma-layout grid
C_WLOCB = C_X0 + 4 * 64  # [128, 32] = 63 bf16 + pad
C_BASISTB = C_WLOCB + 32  # [63p, 64] = 128 bf16
C_SELB = C_BASISTB + 64  # [128, 256] = 8*64 bf16 selectors
C_ESGNB = C_SELB + 256  # [128, 128] = 256 bf16
C_ONESB = C_ESGNB + 128  # [128, 1] = bf16(1/S) packed
C_TW = C_ONESB + 1  # [128, 8] knot in w-coords per (p, ch)
C_SGN = C_TW + 8  # [128, 8] side sign per ch
C_DXT = C_SGN + 8  # [128, 64] Tw - w0 per (p, ch, e)
C_ESGNF = C_DXT + 64  # [128, 256] f32 signed expansion selector (unused)
C_EABSB = C_ESGNF + 256  # [128, 128] = 256 bf16 unsigned expansion selector
CCOLS = C_EABSB + 128

_CACHE = {}


def _build_program():
    import concourse.bass as bass
    import concourse.bacc as bacc
    import concourse.tile as tile
    from concourse import mybir

    alu = mybir.AluOpType
    f32 = mybir.dt.float32

    nc = bacc.Bacc("TRN2", target_bir_lowering=False, debug=False, enable_asserts=False)

    bf16 = mybir.dt.bfloat16

    def mm(out, lhsT, rhs, start=True, stop=True):
        nc.tensor.matmul(out, lhsT, rhs, start=start, stop=stop)

    seq = nc.dram_tensor("seq", [R, S, D], f32, kind="ExternalInput").ap()
    cbd = nc.dram_tensor("cb", [128, CCOLS], f32, kind="ExternalInput").ap()
    gamma = nc.dram_tensor("gamma", [R, S], f32, kind="ExternalOutput").ap()

    with tile.TileContext(nc) as tc:
        with (
            tc.tile_pool(name="const", bufs=1) as p_const,
            tc.tile_pool(name="seqp", bufs=12) as p_seq,
            tc.tile_pool(name="redp", bufs=3) as p_red,
            tc.tile_pool(name="meanps", bufs=1, space=bass.MemorySpace.PSUM) as p_mps,
            tc.tile_pool(name="passps", bufs=1, space=bass.MemorySpace.PSUM) as p_pps,
            tc.tile_pool(name="cwtps", bufs=1, space=bass.MemorySpace.PSUM) as p_cps,
            tc.tile_pool(name="sb", bufs=1) as p_sb,
            tc.tile_pool(name="tbl", bufs=1) as p_tbl,
            tc.tile_pool(name="integ", bufs=2) as p_int,
        ):
            cb = p_const.tile([128, CCOLS], f32, tag="cb")
            nc.scalar.dma_start(cb[:], cbd)
            wloc = cb[:, C_WLOCB : C_WLOCB + 32].bitcast(bf16)[:, 0:DTH]
            basisT = cb[0:DTH, C_BASISTB : C_BASISTB + 64].bitcast(bf16)
            sel_bf = cb[:, C_SELB : C_SELB + 256].bitcast(bf16)
            esgn_bf = cb[:, C_ESGNB : C_ESGNB + 128].bitcast(bf16)
            eabs_bf = cb[:, C_EABSB : C_EABSB + 128].bitcast(bf16)
            bloc = cb[0:DTH, C_BLOC : C_BLOC + 1]
            ones_bf = cb[:, C_ONESB : C_ONESB + 1].bitcast(bf16)[:, 0:1]
            w0v = cb[:, C_W0 : C_W0 + 8 * E].rearrange("p (c e) -> p c e", e=E)
            Twv = cb[:, C_TW : C_TW + 8].rearrange("p (c o) -> p c o", o=1)
            sgnv = cb[:, C_SGN : C_SGN + 8].rearrange("p (c o) -> p c o", o=1)
            dxTv = cb[:, C_DXT : C_DXT + 64].rearrange("p (c e) -> p c e", e=E)
            x0v = cb[:, C_X0 : C_X0 + 4 * 64].rearrange("p (c j) -> p c j", j=64)

            ACT_COPY = mybir.ActivationFunctionType.Copy

            mean_ps = p_mps.tile([128, R], f32, tag="meanps")
            mean_sb = p_sb.tile([128, R], bf16, tag="mean")
            # expanded per-(row,cell,side) tables in edge layout:
            # cols 0 a | 1 a' | 2 G=e^a | 3 sigma*b | 4 sigma*b' | 5 H=sigma*h
            cwt_all = p_sb.tile([128, 8, 6], f32, tag="cwtall")
            cw_ps_all = p_cps.tile([128, 8, 64], f32, tag="cwps")  # pad to a full 2KB bank

            # issue all half-row DMAs upfront (contiguous 8KB/partition each);
            # the pool's WAR tracking paces reuse automatically
            NH = NT // 2  # 16 blocks per half
            seq_halves = []
            for i in range(2 * R):
                t = p_seq.tile([128, NH, D], f32, tag="seq", name=f"seq{i}")
                nc.sync.dma_start(
                    t[:],
                    seq[i // 2].rearrange("(h p n) d -> h p n d", h=2, p=128)[i % 2],
                )
                seq_halves.append(t)

            def half_tree(r, h):
                # binary-tree reduce over n on DVE down to [128, 4, D]; the
                # last level converts to bf16 so PE (FWL) finishes the sum
                src = seq_halves[2 * r + h]
                m = NH
                while m > 4:
                    m //= 2
                    dst = p_red.tile(
                        [128, m, D], bf16 if m == 4 else f32,
                        tag=f"red{h}_{m}", name=f"red{m}_{r}_{h}"
                    )
                    nc.vector.tensor_tensor(
                        out=dst[:], in0=src[:, 0:m, :], in1=src[:, m : 2 * m, :],
                        op=alu.add,
                    )
                    src = dst
                return src

            def do_row(r):
                a = half_tree(r, 0)
                b = half_tree(r, 1)
                for j in range(4):
                    mm(mean_ps[:, r : r + 1], a[:, j, :], ones_bf,
                       start=(j == 0), stop=False)
                for j in range(4):
                    mm(mean_ps[:, r : r + 1], b[:, j, :], ones_bf,
                       start=False, stop=(j == 3))
                nc.scalar.copy(mean_sb[:, r : r + 1], mean_ps[:, r : r + 1])

            def do_pass(g):
                # theta & A for rows (2g, 2g+1)
                th_ps = p_pps.tile([DTH, 2], f32, tag="thps", name=f"thps{g}")
                mm(th_ps[:], wloc, mean_sb[:, 2 * g : 2 * g + 2])
                th = p_tbl.tile([DTH, 2], bf16, tag=f"th{g}")
                nc.vector.tensor_scalar(
                    out=th[:], in0=th_ps[:], scalar1=bloc, scalar2=None, op0=alu.add
                )
                ab_ps = p_pps.tile([128, 2], f32, tag="abps", name=f"abps{g}")
                mm(ab_ps[:], basisT, th[:])
                ab = p_tbl.tile([128, 2], bf16, tag=f"ab{g}")
                nc.scalar.copy(ab[:], ab_ps[:])

                # per-(h,c) constants:
                # q = (a_cur, a_cur, a_nxt, a_prv, b_cur, b_cur, b_nxt, b_prv)
                c_ps = p_pps.tile([128, 8], f32, tag="cps", name=f"cps{g}")
                for q in range(8):
                    for h in range(2):
                        mm(
                            c_ps[64 * h : 64 * h + 64, q : q + 1],
                            sel_bf[:, 64 * q : 64 * q + 64],
                            ab[:, h : h + 1],
                        )
                cons = p_tbl.tile([128, 8], f32, tag=f"cons{g}")
                nc.scalar.copy(cons[:], c_ps[:])
                a_cur, b_cur = cons[:, 0:1], cons[:, 4:5]

                # TBf: eabs cols 0:6 = (a,a,a',a'_other,G,G),
                #      esgn cols 6:12 = (b,b,b'n,b'p,h,h); stride-2 side picks
                TBf = p_tbl.tile([128, 12], bf16, tag=f"TBf{g}")
                nc.scalar.copy(TBf[:, 0:4], cons[:, 0:4])
                nc.scalar.copy(TBf[:, 6:10], cons[:, 4:8])
                # E(a) = (e^a - 1)/a = 1 + a(1/2 + a(1/6 + a/24)); g = 1 + aE, h = bE
                ep = p_tbl.tile([128, 2], f32, tag=f"ep{g}")
                nc.vector.tensor_scalar(
                    out=ep[:, 0:1], in0=a_cur, scalar1=float(1.0 / 24.0),
                    scalar2=float(1.0 / 6.0), op0=alu.mult, op1=alu.add,
                )
                nc.vector.tensor_scalar(
                    out=ep[:, 1:2], in0=ep[:, 0:1], scalar1=a_cur,
                    scalar2=0.5, op0=alu.mult, op1=alu.add,
                )
                Ea = p_tbl.tile([128, 1], f32, tag=f"Ea{g}")
                nc.vector.tensor_scalar(
                    out=Ea[:], in0=ep[:, 1:2], scalar1=a_cur,
                    scalar2=1.0, op0=alu.mult, op1=alu.add,
                )
                nc.vector.tensor_scalar(
                    out=TBf[:, 4:5], in0=Ea[:], scalar1=a_cur, scalar2=None,
                    op0=alu.mult,
                )  # G-1 = a*E(a): bf16-safe (small), reconstruct G-1+1 downstream
                nc.scalar.copy(TBf[:, 5:6], TBf[:, 4:5])
                nc.vector.tensor_scalar(
                    out=TBf[:, 10:11], in0=Ea[:], scalar1=b_cur, scalar2=None,
                    op0=alu.mult,
                )
                nc.scalar.copy(TBf[:, 11:12], TBf[:, 10:11])

                # expansion into edge layout (PE), directly at partition 32g
                cw_ps = cw_ps_all[32 * g : 32 * g + 32]
                for ch in range(8):
                    side = ch % 2  # 0=L, 1=R
                    base = 1 - side
                    nc.tensor.matmul(
                        cw_ps[:, ch, 0:3],
                        eabs_bf[:, 32 * ch : 32 * ch + 32],
                        TBf[:, 0:6].rearrange("p (a b) -> p a b", b=2)[
                            :, :, base : base + 1
                        ],
                        start=True, stop=True, tile_position=(0, 32 * g),
                    )
                    nc.tensor.matmul(
                        cw_ps[:, ch, 3:6],
                        esgn_bf[:, 32 * ch : 32 * ch + 32],
                        TBf[:, 6:12].rearrange("p (a b) -> p a b", b=2)[
                            :, :, base : base + 1
                        ],
                        start=True, stop=True, tile_position=(0, 32 * g),
                    )
                nc.scalar.copy(cwt_all[32 * g : 32 * g + 32, :, :], cw_ps[:, :, 0:6])

            for r in range(R):
                do_row(r)
                if r % 2 == 1:
                    do_pass(r // 2)

            # ---- bulk finals: x = x0 + ((G-1)*x0 + H) ----
            Gm1b = cwt_all[:, 1::2, 2:3]  # [128, 4, 1] (side-independent)
            Hb = cwt_all[:, 1::2, 5:6]  # +h on the right-side channels
            xg1 = p_int.tile([128, 4, 64], f32, tag="xg1")
            nc.vector.tensor_tensor(
                out=xg1[:], in0=x0v, in1=Gm1b.broadcast_to([128, 4, 64]), op=alu.mult
            )
            nc.vector.tensor_tensor(
                out=xg1[:], in0=xg1[:], in1=Hb.broadcast_to([128, 4, 64]), op=alu.add
            )
            xg = p_int.tile([128, 4, 64], f32, tag="xg")
            nc.vector.tensor_tensor(
                out=xg[:], in0=xg1[:], in1=x0v, op=alu.add
            )

            # ---- edge points: exact affine-flow closed form in w-coords ----
            # own-cell flow: w_nc = G*w0 + Hw; crossing iff w_nc > Tw;
            # crossing time t* = (dxT/v0)*ln1p(q)/q with q = a*dxT/v0;
            # then x1 = Tw + (1-t*)*v'(Tw)*E(a'*(1-t*)) in the neighbor cell
            SH = [128, 8, E]
            ab_ = cwt_all[:, :, 0:1].broadcast_to(SH)
            apb = cwt_all[:, :, 1:2].broadcast_to(SH)
            Gb2 = cwt_all[:, :, 2:3].broadcast_to(SH)
            bwb = cwt_all[:, :, 3:4].broadcast_to(SH)
            Hwb = cwt_all[:, :, 5:6].broadcast_to(SH)
            Twb = Twv.broadcast_to(SH)
            sgb = sgnv.broadcast_to(SH)

            def etile(nm):
                return p_int.tile(SH, f32, tag=nm, name=nm)

            # v'(Tw) in the neighbor cell, per (p, ch)
            vtp = p_int.tile([128, 8, 1], f32, tag="vtp")
            nc.vector.tensor_tensor(
                out=vtp[:], in0=cwt_all[:, :, 1:2], in1=Twv, op=alu.mult
            )
            nc.vector.tensor_tensor(
                out=vtp[:], in0=vtp[:], in1=cwt_all[:, :, 4:5], op=alu.add
            )
            vtpb = vtp[:].broadcast_to(SH)

            v0 = etile("v0")
            nc.vector.tensor_tensor(out=v0[:], in0=w0v, in1=ab_, op=alu.mult)
            nc.vector.tensor_tensor(out=v0[:], in0=v0[:], in1=bwb, op=alu.add)
            nc.vector.tensor_scalar(
                out=v0[:], in0=v0[:], scalar1=1e-12, scalar2=None, op0=alu.add
            )
            rv = etile("rv")
            nc.vector.reciprocal(rv[:], v0[:])
            nc.vector.tensor_scalar(
                out=rv[:], in0=rv[:], scalar1=1e6, scalar2=-1e6,
                op0=alu.min, op1=alu.max,
            )
            q = etile("q")
            nc.vector.tensor_tensor(out=q[:], in0=rv[:], in1=ab_, op=alu.mult)
            nc.vector.tensor_tensor(out=q[:], in0=q[:], in1=dxTv, op=alu.mult)
            u = etile("u")
            nc.vector.tensor_tensor(out=u[:], in0=rv[:], in1=dxTv, op=alu.mult)
            L = etile("L")
            nc.vector.tensor_scalar(
                out=L[:], in0=q[:], scalar1=float(1.0 / 3.0), scalar2=-0.5,
                op0=alu.mult, op1=alu.add,
            )
            nc.vector.tensor_tensor(out=L[:], in0=L[:], in1=q[:], op=alu.mult)
            nc.vector.tensor_scalar(
                out=L[:], in0=L[:], scalar1=1.0, scalar2=None, op0=alu.add
            )
            tau = etile("tau")
            nc.vector.tensor_tensor(out=tau[:], in0=u[:], in1=L[:], op=alu.mult)
            nc.vector.tensor_scalar(
                out=tau[:], in0=tau[:], scalar1=-1.0, scalar2=1.0,
                op0=alu.mult, op1=alu.add,
            )
            m = etile("m")
            nc.vector.tensor_tensor(out=m[:], in0=tau[:], in1=apb, op=alu.mult)
            Em = etile("Em")
            nc.vector.tensor_scalar(
                out=Em[:], in0=m[:], scalar1=float(1.0 / 6.0), scalar2=0.5,
                op0=alu.mult, op1=alu.add,
            )
            nc.vector.tensor_tensor(out=Em[:], in0=Em[:], in1=m[:], op=alu.mult)
            nc.vector.tensor_scalar(
                out=Em[:], in0=Em[:], scalar1=1.0, scalar2=None, op0=alu.add
            )
            xc = etile("xc")
            nc.vector.tensor_tensor(out=xc[:], in0=tau[:], in1=vtpb, op=alu.mult)
            nc.vector.tensor_tensor(out=xc[:], in0=xc[:], in1=Em[:], op=alu.mult)
            nc.vector.tensor_tensor(out=xc[:], in0=xc[:], in1=Twb, op=alu.add)
            xnc = etile("xnc")
            nc.vector.tensor_tensor(out=xnc[:], in0=w0v, in1=Gb2, op=alu.mult)
            nc.vector.tensor_tensor(out=xnc[:], in0=xnc[:], in1=Hwb, op=alu.add)
            nc.vector.tensor_tensor(out=xnc[:], in0=xnc[:], in1=w0v, op=alu.add)
            sel = etile("sel")
            nc.vector.tensor_tensor(out=sel[:], in0=xnc[:], in1=Twb, op=alu.is_gt)
            dd = etile("dd")
            nc.vector.tensor_tensor(out=dd[:], in0=xc[:], in1=xnc[:], op=alu.subtract)
            nc.vector.tensor_tensor(out=dd[:], in0=dd[:], in1=sel[:], op=alu.mult)
            xe = etile("xe")
            nc.vector.tensor_tensor(out=xe[:], in0=xnc[:], in1=dd[:], op=alu.add)
            nc.vector.tensor_tensor(out=xe[:], in0=xe[:], in1=sgb, op=alu.mult)

            # merge edges into the gamma tile
            nc.vector.tensor_copy(xg[:, :, 0:E], xe[:, 0:8:2, :])
            nc.vector.tensor_copy(xg[:, :, 64 - E : 64], xe[:, 1:8:2, :])
            nc.sync.dma_start(
                gamma.rearrange("r (cq f) -> (r cq) f", f=256), xg[:]
            )

    nc.compile()
    return nc


def _pack_bf16(a):
    """(P, n) float -> (P, ceil(n/2)) float32 words holding packed bf16 pairs."""
    v = np.ascontiguousarray(a, dtype=np.float32).view(np.uint32)
    # round-to-nearest-even truncation to bf16
    a16 = ((v + 0x7FFF + ((v >> 16) & 1)) >> 16).astype(np.uint16)
    P, n = a16.shape
    if n % 2:
        a16 = np.concatenate([a16, np.zeros((P, 1), np.uint16)], axis=1)
    u32 = a16[:, 0::2].astype(np.uint32) | (a16[:, 1::2].astype(np.uint32) << 16)
    return u32.view(np.float32)


def _host_constants():
    f32 = np.float32
    grid = np.linspace(0.0, 1.0, S).astype(f32)
    cbk = np.zeros((128, CCOLS), dtype=f32)
    c = np.arange(128, dtype=np.int64) % 64
    cbk[:, C_ONES] = 1.0 / S  # 2^-12, exact
    cbk[:, C_ONESB : C_ONESB + 1] = _pack_bf16(np.full((128, 1), 1.0 / S, f32))
    cbk[:, C_TKP] = ((c + 1) / 64.0).astype(f32)
    cbk[:, C_TKM] = (c / 64.0).astype(f32)
    # selectors: row k = flat A index, col = q*64 + cell
    # q = (a_cur, a_cur, a_nxt, a_prv, b_cur, b_cur, b_nxt, b_prv)
    sel = np.zeros((128, 8 * 64), dtype=f32)
    cc = np.arange(64)
    sel[2 * cc, 0 * 64 + cc] = 1.0
    sel[2 * cc, 1 * 64 + cc] = 1.0
    sel[np.minimum(2 * cc + 2, 126), 2 * 64 + cc] = 1.0  # a_nxt (c=63 -> self)
    sel[np.maximum(2 * cc - 2, 0), 3 * 64 + cc] = 1.0  # a_prv (c=0 -> self)
    sel[2 * cc + 1, 4 * 64 + cc] = 1.0
    sel[2 * cc + 1, 5 * 64 + cc] = 1.0
    sel[np.minimum(2 * cc + 3, 127), 6 * 64 + cc] = 1.0  # b_nxt (c=63 -> self)
    sel[np.maximum(2 * cc - 1, 1), 7 * 64 + cc] = 1.0  # b_prv (c=0 -> self)
    cbk[:, C_SELB : C_SELB + 256] = _pack_bf16(sel)
    # expansion selectors: k = h*64 + c (pass layout), m = 16*h + cq (local)
    esgn = np.zeros((128, 8 * 32), dtype=f32)
    eabs = np.zeros((128, 8 * 32), dtype=f32)
    for ch in range(8):
        c4, side = ch // 2, ch % 2
        sgn = -1.0 if side == 0 else 1.0
        for m in range(32):
            h, cq = m // 16, m % 16
            k = h * 64 + 4 * cq + c4
            esgn[k, 32 * ch + m] = sgn
            eabs[k, 32 * ch + m] = 1.0
    cbk[:, C_ESGNB : C_ESGNB + 128] = _pack_bf16(esgn)
    cbk[:, C_EABSB : C_EABSB + 128] = _pack_bf16(eabs)
    cbk[:, C_EABS : C_EABS + 256] = eabs
    # w0[p, ch, e]: p = 16r + cq, ch = (c4, side); L: -grid[64c+e], R: grid[64c+56+e]
    w0map = np.zeros((128, 8, E), dtype=f32)
    for p in range(128):
        cq = p % 16
        for ch in range(8):
            c4, side = ch // 2, ch % 2
            cell = 4 * cq + c4
            if side == 0:
                w0map[p, ch, :] = -grid[64 * cell : 64 * cell + E]
            else:
                w0map[p, ch, :] = grid[64 * cell + 64 - E : 64 * cell + 64]
    cbk[:, C_W0 : C_W0 + 64] = w0map.reshape(128, 64)
    # knot in w-coords, side sign, and Tw - w0 per (p, ch[, e])
    Tw = np.zeros((128, 8), dtype=f32)
    sgn = np.zeros((128, 8), dtype=f32)
    for p in range(128):
        cq = p % 16
        for ch in range(8):
            c4, side = ch // 2, ch % 2
            cell = 4 * cq + c4
            if side == 0:
                Tw[p, ch] = -cell / 64.0
                sgn[p, ch] = -1.0
            else:
                Tw[p, ch] = (cell + 1) / 64.0
                sgn[p, ch] = 1.0
    cbk[:, C_TW : C_TW + 8] = Tw
    cbk[:, C_SGN : C_SGN + 8] = sgn
    cbk[:, C_DXT : C_DXT + 64] = (Tw[:, :, None] - w0map).reshape(128, 64)
    # x0 in gamma layout: [p=(r,cq), c4, j] = grid[256*cq + 64*c4 + j]
    cq = np.arange(128) % 16
    x0g = grid[
        (256 * cq)[:, None, None]
        + (64 * np.arange(4))[None, :, None]
        + np.arange(64)[None, None, :]
    ]
    cbk[:, C_X0 : C_X0 + 256] = x0g.reshape(128, 256)
    return cbk


def _in_map(input_seq_slice, W_loc, b_loc, basis, cbk):
    f32 = np.float32
    cbk = cbk.copy()
    cbk[:, C_WLOCB : C_WLOCB + 32] = _pack_bf16(np.asarray(W_loc, dtype=f32))
    cbk[0:DTH, C_BASISTB : C_BASISTB + 64] = _pack_bf16(
        np.asarray(basis, dtype=f32).T
    )
    cbk[0:DTH, C_BLOC] = np.asarray(b_loc, dtype=f32)
    return {
        "seq": np.ascontiguousarray(input_seq_slice, dtype=f32),
        "cb": cbk,
    }


def kernel(input_seq, W_loc, b_loc, basis):
    from concourse.bass_utils import run_bass_kernel_spmd

    if "nc" not in _CACHE:
        _CACHE["nc"] = _build_program()
    nc = _CACHE["nc"]
    cbk = _host_constants()
    in_maps = [
        _in_map(input_seq[k * R : (k + 1) * R], W_loc, b_loc, basis, cbk)
        for k in range(NCORES)
    ]
    res = run_bass_kernel_spmd(nc, in_maps, core_ids=list(range(NCORES)))
    return np.concatenate([r["gamma"] for r in res.results], axis=0)
